# revision 6
# baseline (speedup 1.0000x reference)
"""GNN message-passing (MetaLayer-style) Trainium2 kernel, 8 NeuronCores SPMD.

Sharding: destination-node partitioning (6250 nodes/core). Per layer:
  zr = relu(x @ n1w1 + b1) computed per-shard, AllGathered to form the full
  gather table; edge aggregation = dma_gather(zr[row]) + one-hot-matmul
  segment-sum by dest; node MLP + global MLP fused on-device; u partial sums
  AllReduced. All 4 layers run in ONE kernel launch.

Key algebraic transforms vs the reference:
  - x[row] @ n1w1 == (x @ n1w1)[row]  -> per-node MLP (12x fewer flops)
  - scatter_mean(relu(z)[row] @ n1w2 + b2) ==
      (scatter_sum(relu(z)[row]) / cnt) @ n1w2 + b2*(cnt>0)
"""
import sys

sys.path.insert(0, "/opt/trn_rl_repo")
import numpy as np

P = 128
N = 50000
F = H = U = 128
E = 600000
B = 64
L = 4
NCORE = 8
NS = N // NCORE          # 6250 nodes per core
HALF = 25000             # int16 gather index reach split
NDT = 49                 # dest tiles per core (48*128 + 106)
LAST_W = NS - 48 * P     # 106
SUPERS = [(s * 4, min(4, NDT - s * 4)) for s in range((NDT + 3) // 4)]  # 13

_BUILD_CACHE = {}
LAST_RESULT = None
LAST_EXEC_NS = None


def _build(T):
    import concourse.bass as bass
    import concourse.bacc as bacc
    import concourse.mybir as mybir
    import concourse.tile as tile
    from concourse import library_config

    f32 = mybir.dt.float32
    AF = mybir.ActivationFunctionType
    NT = NDT * 2 * T               # K-tiles per core

    nc = bacc.Bacc("TRN2", target_bir_lowering=False, debug=False,
                   num_devices=NCORE)
    dram_in = lambda nm, sh, dt=f32: nc.dram_tensor(nm, sh, dt, kind="ExternalInput")
    xT_in = dram_in("xT_in", [P, NS])
    uT_in = dram_in("uT_in", [P, B])
    idx_in = dram_in("idx_in", [P, NT * 8], mybir.dt.int16)
    rc_in = dram_in("rc_in", [P, NT])
    invc_in = dram_in("invc_in", [P, NS])
    mask_in = dram_in("mask_in", [1, NS])
    gb_in = dram_in("gb_in", [P, NDT * B])
    gbt_in = dram_in("gbt_in", [B, NS])
    invb_in = dram_in("invb_in", [P, B])
    iota_in = dram_in("iota_in", [P, 8, P])
    ones_in = dram_in("ones_in", [1, P])
    ident_in = dram_in("ident_in", [P, P])
    n1w1_d = dram_in("n1w1_d", [L, P, H])
    n1b1_d = dram_in("n1b1_d", [L, 1, H])
    n1w2_d = dram_in("n1w2_d", [L, H, H])
    n1b2_d = dram_in("n1b2_d", [L, 1, H])
    n2w1_d = dram_in("n2w1_d", [L, 3 * P, H])
    n2b1_d = dram_in("n2b1_d", [L, H, 1])
    n2w2_d = dram_in("n2w2_d", [L, H, F])
    n2b2c_d = dram_in("n2b2c_d", [L, F, 1])
    n2b2r_d = dram_in("n2b2r_d", [L, 1, F])
    gw1_d = dram_in("gw1_d", [L, 2 * P, H])
    gb1_d = dram_in("gb1_d", [L, H, 1])
    gw2_d = dram_in("gw2_d", [L, H, U])
    gb2c_d = dram_in("gb2c_d", [L, U, 1])
    gb2r_d = dram_in("gb2r_d", [L, 1, U])
    x_out = nc.dram_tensor("x_out", [NS, F], f32, kind="ExternalOutput")
    u_out = nc.dram_tensor("u_out", [B, U], f32, kind="ExternalOutput")

    with tile.TileContext(nc) as tc:
        with (
            tc.tile_pool(name="res", bufs=1) as res,      # residents
            tc.tile_pool(name="wts", bufs=2) as wts,      # per-layer weights
            tc.tile_pool(name="sb", bufs=2) as sb,        # streaming tiles
            tc.tile_pool(name="gp", bufs=2) as gp,        # gather tiles
            tc.tile_pool(name="ps", bufs=1, space="PSUM") as ps1,
            tc.tile_pool(name="ps2", bufs=2, space="PSUM") as ps2,
            tc.tile_pool(name="dram", bufs=2, space="DRAM") as dram,
        ):
            nc.gpsimd.load_library(library_config.mlp)

            def load(pool, src, sh, dt=f32, tag=None):
                t = pool.tile(sh, dt, tag=tag)
                nc.sync.dma_start(t[:], src)
                return t

            xT = load(res, xT_in[:], [P, NS], tag="xT")
            uT = load(res, uT_in[:], [P, B], tag="uT")
            idx = load(res, idx_in[:], [P, NT * 8], mybir.dt.int16, tag="idx")
            rc = load(res, rc_in[:], [P, NT], tag="rc")
            invc = load(res, invc_in[:], [P, NS], tag="invc")
            gb = load(res, gb_in[:], [P, NDT * B], tag="gb")
            gbt = load(res, gbt_in[:], [B, NS], tag="gbt")
            invb = load(res, invb_in[:], [P, B], tag="invb")
            iota8 = load(res, iota_in[:], [P, 8, P], tag="iota8")
            ones = load(res, ones_in[:], [1, P], tag="ones")
            ident = load(res, ident_in[:], [P, P], tag="ident")
            q = res.tile([B, H], f32, tag="q")

            # initial q = u0 @ n2w1[0][256:384]
            w1u0 = load(wts, n2w1_d[0, 2 * P:3 * P, :], [P, H], tag="w1u")
            psq0 = ps1.tile([B, H], f32, tag="us")
            nc.tensor.matmul(out=psq0[:], lhsT=uT[:], rhs=w1u0[:],
                             start=True, stop=True)
            nc.vector.tensor_copy(out=q[:], in_=psq0[:])

            for l in range(L):
                n1w1 = load(wts, n1w1_d[l], [P, H], tag="n1w1")
                n1b1r = load(wts, n1b1_d[l], [1, H], tag="n1b1r")
                n1w2 = load(wts, n1w2_d[l], [H, H], tag="n1w2")
                n1b2r = load(wts, n1b2_d[l], [1, H], tag="n1b2r")
                w1x = load(wts, n2w1_d[l, 0:P, :], [P, H], tag="w1xa")
                w1a = load(wts, n2w1_d[l, P:2 * P, :], [P, H], tag="w1xb")
                n2b1c = load(wts, n2b1_d[l], [H, 1], tag="n2b1c")
                n2w2 = load(wts, n2w2_d[l], [H, F], tag="n2w2")
                n2b2c = load(wts, n2b2c_d[l], [F, 1], tag="n2b2c")
                n2b2r = load(wts, n2b2r_d[l], [1, F], tag="n2b2r")
                gw1u = load(wts, gw1_d[l, 0:P, :], [P, H], tag="gw1u")
                gw1m = load(wts, gw1_d[l, P:2 * P, :], [P, H], tag="gw1m")
                gb1c = load(wts, gb1_d[l], [H, 1], tag="gb1c")
                gw2 = load(wts, gw2_d[l], [H, U], tag="gw2")
                gb2c = load(wts, gb2c_d[l], [U, 1], tag="gb2c")
                gb2r = load(wts, gb2r_d[l], [1, U], tag="gb2r")

                # ---- phase Z: zr shard = relu(x @ n1w1 + b1), node-major
                zr_shard = dram.tile([NS, H], f32, tag="zr_shard")
                for t in range(NDT):
                    nn = P if t < NDT - 1 else LAST_W
                    psz = ps1.tile([P, H], f32, tag="xnpz")
                    nc.tensor.matmul(out=psz[:nn, :],
                                     lhsT=xT[:, t * P:t * P + nn],
                                     rhs=n1w1[:], start=True, stop=False)
                    nc.tensor.matmul(out=psz[:nn, :], lhsT=ones[:1, :nn],
                                     rhs=n1b1r[:], start=False, stop=True,
                                     skip_group_check=True)
                    zt = sb.tile([P, H], f32, tag="zt")
                    nc.scalar.activation(zt[:nn, :], psz[:nn, :], AF.Relu)
                    nc.sync.dma_start(zr_shard[t * P:t * P + nn, :], zt[:nn, :])

                # ---- allgather zr -> full table
                zr_full = dram.tile([N, H], f32, tag="zr_full")
                nc.gpsimd.collective_compute(
                    "AllGather", mybir.AluOpType.bypass,
                    replica_groups=[list(range(NCORE))],
                    ins=[zr_shard.opt()], outs=[zr_full.opt()],
                )

                # ---- phases S+N per super
                psu = ps1.tile([B, F], f32, tag="us")
                tt = 0  # global K-tile counter
                for si, (d0, ndt) in enumerate(SUPERS):
                    w = ndt * P if d0 + ndt < NDT else (ndt - 1) * P + LAST_W
                    pss = ps2.tile([P, 4 * P], f32, tag="ssum")
                    blk = []
                    for h in range(2):
                        nidx = ndt * T * P
                        gth = gp.tile([P, 4 * T, P], f32, tag="gath")
                        tab = zr_full[0:HALF, :] if h == 0 else zr_full[HALF:N, :]
                        nc.gpsimd.dma_gather(
                            gth[:, :ndt * T, :], tab,
                            idx[:, tt * 8:(tt + ndt * T) * 8],
                            nidx, nidx, H, single_packet=False)
                        blk.append((gth, tt))
                        tt += ndt * T
                    for h in range(2):
                        gth, t0 = blk[h]
                        j = 0
                        while j < ndt * T:
                            g = min(8, ndt * T - j)
                            s8 = sb.tile([P, 8, P], f32, tag="s8")
                            nc.vector.tensor_tensor(
                                out=s8[:, :g, :],
                                in0=rc[:, t0 + j:t0 + j + g, None].broadcast_to([P, g, P]),
                                in1=iota8[:, :g, :],
                                op=mybir.AluOpType.is_equal)
                            for i in range(g):
                                dtl = (j + i) // T
                                ii = (j + i) % T
                                nc.tensor.matmul(
                                    out=pss[:, dtl * P:(dtl + 1) * P],
                                    lhsT=gth[:, j + i, :], rhs=s8[:, i, :],
                                    start=(h == 0 and j + i == 0),
                                    stop=(h == 1 and j + i == ndt * T - 1),
                                    skip_group_check=True)
                            j += g
                    mean = sb.tile([P, 4 * P], f32, tag="mean")
                    nc.vector.tensor_tensor(out=mean[:, :w], in0=pss[:, :w],
                                            in1=invc[:, d0 * P:d0 * P + w],
                                            op=mybir.AluOpType.mult)
                    psa = ps1.tile([P, 4 * P], f32, tag="agg")
                    nc.tensor.matmul(out=psa[:, :w], lhsT=n1w2[:],
                                     rhs=mean[:, :w], start=True, stop=False)
                    mrow = sb.tile([1, 4 * P], f32, tag="mrow")
                    nc.sync.dma_start(mrow[:1, :w],
                                      mask_in[:1, d0 * P:d0 * P + w])
                    nc.tensor.matmul(out=psa[:, :w], lhsT=n1b2r[:1, :],
                                     rhs=mrow[:1, :w],
                                     start=False, stop=True,
                                     skip_group_check=True)
                    aggs = sb.tile([P, 4 * P], f32, tag="aggs")
                    nc.vector.tensor_copy(out=aggs[:, :w], in_=psa[:, :w])

                    # node MLP for this chunk
                    psh = ps2.tile([P, 4 * P], f32, tag="h1")
                    nc.tensor.matmul(out=psh[:, :w], lhsT=w1x[:],
                                     rhs=xT[:, d0 * P:d0 * P + w],
                                     start=True, stop=False)
                    nc.tensor.matmul(out=psh[:, :w], lhsT=w1a[:],
                                     rhs=aggs[:, :w], start=False, stop=False,
                                     skip_group_check=True)
                    nc.tensor.matmul(out=psh[:, :w], lhsT=q[:],
                                     rhs=gbt[:, d0 * P:d0 * P + w],
                                     start=False, stop=True,
                                     skip_group_check=True)
                    h1r = sb.tile([P, 4 * P], f32, tag="h1r")
                    nc.scalar.activation(h1r[:, :w], psh[:, :w], AF.Relu,
                                         bias=n2b1c[:])
                    psx = ps1.tile([P, 4 * P], f32, tag="xnT")
                    nc.tensor.matmul(out=psx[:, :w], lhsT=n2w2[:],
                                     rhs=h1r[:, :w], start=True, stop=True)
                    nc.scalar.activation(xT[:, d0 * P:d0 * P + w], psx[:, :w],
                                         AF.Identity, bias=n2b2c[:])
                    for sub in range(ndt):
                        t = d0 + sub
                        nn = P if t < NDT - 1 else LAST_W
                        psn = ps1.tile([P, F], f32, tag="xnpz")
                        nc.tensor.matmul(out=psn[:nn, :],
                                         lhsT=h1r[:, sub * P:sub * P + nn],
                                         rhs=n2w2[:], start=True, stop=False)
                        nc.tensor.matmul(out=psn[:nn, :], lhsT=ones[:1, :nn],
                                         rhs=n2b2r[:], start=False, stop=True,
                                         skip_group_check=True)
                        xn = sb.tile([P, F], f32, tag="xn")
                        nc.vector.tensor_copy(out=xn[:nn, :], in_=psn[:nn, :])
                        nc.tensor.matmul(out=psu[:],
                                         lhsT=gb[:nn, t * B:(t + 1) * B],
                                         rhs=xn[:nn, :], start=(t == 0),
                                         stop=(t == NDT - 1),
                                         skip_group_check=True)
                        if l == L - 1:
                            nc.sync.dma_start(x_out[t * P:t * P + nn, :],
                                              xn[:nn, :])

                # ---- phase U: global MLP
                uss = sb.tile([B, F], f32, tag="uss")
                nc.vector.tensor_copy(out=uss[:], in_=psu[:])
                ar_in = dram.tile([B, F], f32, tag="ar_in")
                ar_out = dram.tile([B, F], f32, tag="ar_out")
                nc.sync.dma_start(ar_in[:], uss[:])
                nc.gpsimd.collective_compute(
                    "AllReduce", mybir.AluOpType.add,
                    replica_groups=[list(range(NCORE))],
                    ins=[ar_in.opt()], outs=[ar_out.opt()],
                )
                usr = sb.tile([B, F], f32, tag="usr")
                nc.sync.dma_start(usr[:], ar_out[:])
                pst = ps1.tile([P, 4 * P], f32, tag="xnT")
                nc.tensor.transpose(out=pst[:, :B], in_=usr[:],
                                    identity=ident[:B, :B])
                umT = sb.tile([P, B], f32, tag="umT")
                nc.vector.tensor_tensor(out=umT[:], in0=pst[:, :B],
                                        in1=invb[:], op=mybir.AluOpType.mult)
                psg = ps2.tile([P, 4 * P], f32, tag="h1")
                nc.tensor.matmul(out=psg[:, :B], lhsT=gw1u[:], rhs=uT[:],
                                 start=True, stop=False)
                nc.tensor.matmul(out=psg[:, :B], lhsT=gw1m[:], rhs=umT[:],
                                 start=False, stop=True, skip_group_check=True)
                ghr = sb.tile([P, B], f32, tag="ghr")
                nc.scalar.activation(ghr[:, :], psg[:, :B], AF.Relu,
                                     bias=gb1c[:])
                psn2 = ps1.tile([P, F], f32, tag="xnpz")
                nc.tensor.matmul(out=psn2[:B, :], lhsT=ghr[:], rhs=gw2[:],
                                 start=True, stop=False)
                nc.tensor.matmul(out=psn2[:B, :], lhsT=ones[:1, :B],
                                 rhs=gb2r[:], start=False, stop=True,
                                 skip_group_check=True)
                if l == L - 1:
                    un = sb.tile([B, U], f32, tag="un")
                    nc.vector.tensor_copy(out=un[:], in_=psn2[:B, :])
                    nc.sync.dma_start(u_out[:], un[:])
                psut = ps1.tile([P, 4 * P], f32, tag="agg")
                nc.tensor.matmul(out=psut[:, :B], lhsT=gw2[:], rhs=ghr[:],
                                 start=True, stop=True)
                nc.scalar.activation(uT[:, :], psut[:, :B], AF.Identity,
                                     bias=gb2c[:])
                if l < L - 1:
                    w1un = load(wts, n2w1_d[l + 1, 2 * P:3 * P, :], [P, H],
                                tag="w1u")
                    psq = ps1.tile([B, H], f32, tag="us")
                    nc.tensor.matmul(out=psq[:], lhsT=uT[:], rhs=w1un[:],
                                     start=True, stop=True)
                    nc.vector.tensor_copy(out=q[:], in_=psq[:])
    nc.compile()
    return nc


def _prep_core(row, col, k, T):
    m = (col >= k * NS) & (col < (k + 1) * NS)
    er = row[m].astype(np.int64)
    ec = (col[m] - k * NS).astype(np.int64)
    idx_parts, rc_parts = [], []
    for d in range(NDT):
        dm = ec >> 7 == d
        erd, ecd = er[dm], ec[dm]
        for h in range(0):
            pass
    # build per (super, half, dtile) in stream order
    idx_stream = np.zeros(NDT * 2 * T * P, np.int16)
    rc_stream = np.full(NDT * 2 * T * P, -1.0, np.float32)
    pos = 0
    for d0, ndt in SUPERS:
        for h in range(2):
            for d in range(d0, d0 + ndt):
                dm = (ec >> 7) == d
                erd, ecd = er[dm], ec[dm]
                hm = erd < HALF if h == 0 else erd >= HALF
                ei = erd[hm] - (0 if h == 0 else HALF)
                ri = (ecd[hm] & 127).astype(np.float32)
                n = len(ei)
                assert n <= T * P, (k, d, h, n)
                idx_stream[pos:pos + n] = ei.astype(np.int16)
                rc_stream[pos:pos + n] = ri
                pos += T * P
    NT = NDT * 2 * T
    wrap = idx_stream.reshape(NT * 8, 16).T  # [16, NT*8]
    idx_wrap = np.tile(wrap, (8, 1)).astype(np.int16)  # [128, NT*8]
    rc_T = np.ascontiguousarray(rc_stream.reshape(NT, P).T)  # [128, NT]
    cnt = np.bincount(ec, minlength=NS).astype(np.float32)
    invc = np.tile((1.0 / np.maximum(cnt, 1.0))[None, :], (P, 1))
    maskr = (cnt > 0).astype(np.float32)[None, :]
    return idx_wrap, rc_T, invc, maskr


def kernel(x, edge_index, u, batch,
           n1w1, n1b1, n1w2, n1b2, n2w1, n2b1, n2w2, n2b2,
           gw1, gb1, gw2, gb2):
    from concourse.bass_utils import run_bass_kernel_spmd

    x = np.asarray(x, np.float32)
    u = np.asarray(u, np.float32)
    row = np.asarray(edge_index[0], np.int64)
    col = np.asarray(edge_index[1], np.int64)
    batch_np = np.asarray(batch, np.int64)
    prm = {k: np.asarray(v, np.float32) for k, v in dict(
        n1w1=n1w1, n1b1=n1b1, n1w2=n1w2, n1b2=n1b2, n2w1=n2w1, n2b1=n2b1,
        n2w2=n2w2, n2b2=n2b2, gw1=gw1, gb1=gb1, gw2=gw2, gb2=gb2).items()}

    # uniform K-tiles per (core, dtile, half)
    T = 1
    for k in range(NCORE):
        m = (col >= k * NS) & (col < (k + 1) * NS)
        ec = col[m] - k * NS
        er = row[m]
        for h in range(2):
            hm = er < HALF if h == 0 else er >= HALF
            c = np.bincount(ec[hm] >> 7, minlength=NDT)
            T = max(T, int(-(-c.max() // P)))

    if T not in _BUILD_CACHE:
        _BUILD_CACHE[T] = _build(T)
    nc = _BUILD_CACHE[T]

    bcnt = np.bincount(batch_np, minlength=B).astype(np.float32)
    invb = np.tile((1.0 / np.maximum(bcnt, 1.0))[None, :], (P, 1))
    iota = np.tile(np.arange(P, dtype=np.float32), (P, 8, 1))
    common = dict(
        uT_in=np.ascontiguousarray(u.T), invb_in=invb, iota_in=iota,
        ones_in=np.ones((1, P), np.float32),
        ident_in=np.eye(P, dtype=np.float32),
        n1w1_d=prm["n1w1"], n1b1_d=prm["n1b1"][:, None, :],
        n1w2_d=prm["n1w2"], n1b2_d=prm["n1b2"][:, None, :],
        n2w1_d=prm["n2w1"], n2b1_d=prm["n2b1"][:, :, None],
        n2w2_d=prm["n2w2"], n2b2c_d=prm["n2b2"][:, :, None],
        n2b2r_d=prm["n2b2"][:, None, :],
        gw1_d=prm["gw1"], gb1_d=prm["gb1"][:, :, None],
        gw2_d=prm["gw2"], gb2c_d=prm["gb2"][:, :, None],
        gb2r_d=prm["gb2"][:, None, :],
    )
    in_maps = []
    for k in range(NCORE):
        idx_wrap, rc_T, invc, maskr = _prep_core(row, col, k, T)
        nsl = slice(k * NS, (k + 1) * NS)
        bloc = batch_np[nsl]
        gb_arr = np.zeros((P, NDT * B), np.float32)
        gbt_arr = np.zeros((B, NS), np.float32)
        for t in range(NDT):
            nn = P if t < NDT - 1 else LAST_W
            bb = bloc[t * P:t * P + nn]
            gb_arr[np.arange(nn), t * B + bb] = 1.0
        gbt_arr[bloc, np.arange(NS)] = 1.0
        in_maps.append(dict(
            xT_in=np.ascontiguousarray(x[nsl].T), idx_in=idx_wrap,
            rc_in=rc_T, invc_in=invc, mask_in=maskr, gb_in=gb_arr,
            gbt_in=gbt_arr, **common))

    global LAST_RESULT, LAST_EXEC_NS
    import time as _t
    _t0 = _t.perf_counter()
    res = run_bass_kernel_spmd(nc, in_maps, core_ids=list(range(NCORE)))
    LAST_EXEC_NS = int((_t.perf_counter() - _t0) * 1e9)
    LAST_RESULT = res
    x_new = np.concatenate([res.results[k]["x_out"] for k in range(NCORE)], 0)
    u_new = res.results[0]["u_out"]
    return (x_new, u_new)


# revision 8
# speedup vs baseline: 1.3230x; 1.3230x over previous
"""GNN message-passing (MetaLayer-style) Trainium2 kernel, 8 NeuronCores SPMD.

Sharding: destination-node partitioning (6250 nodes/core). Per layer:
  zr = relu(x @ n1w1 + b1) computed per-shard, AllGathered to form the full
  gather table; edge aggregation = dma_gather(zr[row]) + one-hot-matmul
  segment-sum by dest; node MLP + global MLP fused on-device; u partial sums
  AllReduced. All 4 layers run in ONE kernel launch.

Key algebraic transforms vs the reference:
  - x[row] @ n1w1 == (x @ n1w1)[row]  -> per-node MLP (12x fewer flops)
  - scatter_mean(relu(z)[row] @ n1w2 + b2) ==
      (scatter_sum(relu(z)[row]) / cnt) @ n1w2 + b2*(cnt>0)
"""
import sys

sys.path.insert(0, "/opt/trn_rl_repo")
import numpy as np

P = 128
N = 50000
F = H = U = 128
E = 600000
B = 64
L = 4
NCORE = 8
NS = N // NCORE          # 6250 nodes per core
HALF = 25000             # int16 gather index reach split
NDT = 49                 # dest tiles per core (48*128 + 106)
LAST_W = NS - 48 * P     # 106
SUPERS = [(s * 4, min(4, NDT - s * 4)) for s in range((NDT + 3) // 4)]  # 13

_BUILD_CACHE = {}
LAST_RESULT = None
LAST_EXEC_NS = None


def _build(T):
    import concourse.bass as bass
    import concourse.bacc as bacc
    import concourse.mybir as mybir
    import concourse.tile as tile
    from concourse import library_config

    f32 = mybir.dt.float32
    AF = mybir.ActivationFunctionType
    NT = NDT * 2 * T               # K-tiles per core

    nc = bacc.Bacc("TRN2", target_bir_lowering=False, debug=False,
                   num_devices=NCORE)
    dram_in = lambda nm, sh, dt=f32: nc.dram_tensor(nm, sh, dt, kind="ExternalInput")
    xT_in = dram_in("xT_in", [P, NS])
    uT_in = dram_in("uT_in", [P, B])
    idx_in = dram_in("idx_in", [P, NT * 8], mybir.dt.int16)
    rc_in = dram_in("rc_in", [P, NT])
    invc_in = dram_in("invc_in", [P, NS])
    mask_in = dram_in("mask_in", [1, NS])
    gb_in = dram_in("gb_in", [P, NDT * B])
    gbt_in = dram_in("gbt_in", [B, NS])
    invb_in = dram_in("invb_in", [P, B])
    iota_in = dram_in("iota_in", [P, 8, P])
    ones_in = dram_in("ones_in", [1, P])
    ident_in = dram_in("ident_in", [P, P])
    n1w1_d = dram_in("n1w1_d", [L, P, H])
    n1b1_d = dram_in("n1b1_d", [L, 1, H])
    n1w2_d = dram_in("n1w2_d", [L, H, H])
    n1b2_d = dram_in("n1b2_d", [L, 1, H])
    n2w1_d = dram_in("n2w1_d", [L, 3 * P, H])
    n2b1_d = dram_in("n2b1_d", [L, H, 1])
    n2w2_d = dram_in("n2w2_d", [L, H, F])
    n2b2c_d = dram_in("n2b2c_d", [L, F, 1])
    n2b2r_d = dram_in("n2b2r_d", [L, 1, F])
    gw1_d = dram_in("gw1_d", [L, 2 * P, H])
    gb1_d = dram_in("gb1_d", [L, H, 1])
    gw2_d = dram_in("gw2_d", [L, H, U])
    gb2c_d = dram_in("gb2c_d", [L, U, 1])
    gb2r_d = dram_in("gb2r_d", [L, 1, U])
    x_out = nc.dram_tensor("x_out", [NS, F], f32, kind="ExternalOutput")
    u_out = nc.dram_tensor("u_out", [B, U], f32, kind="ExternalOutput")

    with tile.TileContext(nc) as tc:
        with (
            tc.tile_pool(name="res", bufs=1) as res,      # residents
            tc.tile_pool(name="wts", bufs=2) as wts,      # per-layer weights
            tc.tile_pool(name="sb", bufs=2) as sb,        # streaming tiles
            tc.tile_pool(name="gp", bufs=2) as gp,        # gather tiles
            tc.tile_pool(name="ps", bufs=1, space="PSUM") as ps1,
            tc.tile_pool(name="ps2", bufs=2, space="PSUM") as ps2,
            tc.tile_pool(name="dram", bufs=2, space="DRAM") as dram,
        ):
            nc.gpsimd.load_library(library_config.mlp)

            def load(pool, src, sh, dt=f32, tag=None):
                t = pool.tile(sh, dt, tag=tag)
                nc.sync.dma_start(t[:], src)
                return t

            xT = load(res, xT_in[:], [P, NS], tag="xT")
            uT = load(res, uT_in[:], [P, B], tag="uT")
            idx = load(res, idx_in[:], [P, NT * 8], mybir.dt.int16, tag="idx")
            rc = load(res, rc_in[:], [P, NT], tag="rc")
            invc = load(res, invc_in[:], [P, NS], tag="invc")
            gb = load(res, gb_in[:], [P, NDT * B], tag="gb")
            gbt = load(res, gbt_in[:], [B, NS], tag="gbt")
            invb = load(res, invb_in[:], [P, B], tag="invb")
            iota8 = load(res, iota_in[:], [P, 8, P], tag="iota8")
            ones = load(res, ones_in[:], [1, P], tag="ones")
            ident = load(res, ident_in[:], [P, P], tag="ident")
            q = res.tile([B, H], f32, tag="q")

            # initial q = u0 @ n2w1[0][256:384]
            w1u0 = load(wts, n2w1_d[0, 2 * P:3 * P, :], [P, H], tag="w1u")
            psq0 = ps1.tile([B, H], f32, tag="us")
            nc.tensor.matmul(out=psq0[:], lhsT=uT[:], rhs=w1u0[:],
                             start=True, stop=True)
            nc.vector.tensor_copy(out=q[:], in_=psq0[:])

            for l in range(L):
                n1w1 = load(wts, n1w1_d[l], [P, H], tag="n1w1")
                n1b1r = load(wts, n1b1_d[l], [1, H], tag="n1b1r")
                n1w2 = load(wts, n1w2_d[l], [H, H], tag="n1w2")
                n1b2r = load(wts, n1b2_d[l], [1, H], tag="n1b2r")
                w1x = load(wts, n2w1_d[l, 0:P, :], [P, H], tag="w1xa")
                w1a = load(wts, n2w1_d[l, P:2 * P, :], [P, H], tag="w1xb")
                n2b1c = load(wts, n2b1_d[l], [H, 1], tag="n2b1c")
                n2w2 = load(wts, n2w2_d[l], [H, F], tag="n2w2")
                n2b2c = load(wts, n2b2c_d[l], [F, 1], tag="n2b2c")
                n2b2r = load(wts, n2b2r_d[l], [1, F], tag="n2b2r")
                gw1u = load(wts, gw1_d[l, 0:P, :], [P, H], tag="gw1u")
                gw1m = load(wts, gw1_d[l, P:2 * P, :], [P, H], tag="gw1m")
                gb1c = load(wts, gb1_d[l], [H, 1], tag="gb1c")
                gw2 = load(wts, gw2_d[l], [H, U], tag="gw2")
                gb2c = load(wts, gb2c_d[l], [U, 1], tag="gb2c")
                gb2r = load(wts, gb2r_d[l], [1, U], tag="gb2r")

                # ---- phase Z: zr shard = relu(x @ n1w1 + b1), node-major
                zr_shard = dram.tile([NS, H], f32, tag="zr_shard")
                for t in range(NDT):
                    nn = P if t < NDT - 1 else LAST_W
                    psz = ps1.tile([P, H], f32, tag="xnpz")
                    nc.tensor.matmul(out=psz[:nn, :],
                                     lhsT=xT[:, t * P:t * P + nn],
                                     rhs=n1w1[:], start=True, stop=False)
                    nc.tensor.matmul(out=psz[:nn, :], lhsT=ones[:1, :nn],
                                     rhs=n1b1r[:], start=False, stop=True,
                                     skip_group_check=True)
                    zt = sb.tile([P, H], f32, tag="zt")
                    nc.scalar.activation(zt[:nn, :], psz[:nn, :], AF.Relu)
                    nc.sync.dma_start(zr_shard[t * P:t * P + nn, :], zt[:nn, :])

                # ---- allgather zr -> full table
                zr_full = dram.tile([N, H], f32, tag="zr_full")
                nc.gpsimd.collective_compute(
                    "AllGather", mybir.AluOpType.bypass,
                    replica_groups=[list(range(NCORE))],
                    ins=[zr_shard.opt()], outs=[zr_full.opt()],
                )

                # ---- phases S+N per super
                psu = ps1.tile([B, F], f32, tag="us")
                tt = 0  # global K-tile counter
                for si, (d0, ndt) in enumerate(SUPERS):
                    w = ndt * P if d0 + ndt < NDT else (ndt - 1) * P + LAST_W
                    pss = ps2.tile([P, 4 * P], f32, tag="ssum")
                    blk = []
                    for h in range(2):
                        nidx = ndt * T * P
                        gth = gp.tile([P, 4 * T, P], f32, tag="gath")
                        tab = zr_full[0:HALF, :] if h == 0 else zr_full[HALF:N, :]
                        nc.gpsimd.dma_gather(
                            gth[:, :ndt * T, :], tab,
                            idx[:, tt * 8:(tt + ndt * T) * 8],
                            nidx, nidx, H, single_packet=False)
                        blk.append((gth, tt))
                        tt += ndt * T
                    for h in range(2):
                        gth, t0 = blk[h]
                        j = 0
                        while j < ndt * T:
                            g = min(8, ndt * T - j)
                            s8 = sb.tile([P, 8, P], f32, tag="s8")
                            nc.vector.tensor_tensor(
                                out=s8[:, :g, :],
                                in0=rc[:, t0 + j:t0 + j + g, None].broadcast_to([P, g, P]),
                                in1=iota8[:, :g, :],
                                op=mybir.AluOpType.is_equal)
                            for i in range(g):
                                dtl = (j + i) // T
                                ii = (j + i) % T
                                nc.tensor.matmul(
                                    out=pss[:, dtl * P:(dtl + 1) * P],
                                    lhsT=gth[:, j + i, :], rhs=s8[:, i, :],
                                    start=(h == 0 and j + i == 0),
                                    stop=(h == 1 and j + i == ndt * T - 1),
                                    skip_group_check=True)
                            j += g
                    mean = sb.tile([P, 4 * P], f32, tag="mean")
                    nc.vector.tensor_tensor(out=mean[:, :w], in0=pss[:, :w],
                                            in1=invc[:, d0 * P:d0 * P + w],
                                            op=mybir.AluOpType.mult)
                    psa = ps1.tile([P, 4 * P], f32, tag="agg")
                    nc.tensor.matmul(out=psa[:, :w], lhsT=n1w2[:],
                                     rhs=mean[:, :w], start=True, stop=False)
                    mrow = sb.tile([1, 4 * P], f32, tag="mrow")
                    nc.sync.dma_start(mrow[:1, :w],
                                      mask_in[:1, d0 * P:d0 * P + w])
                    nc.tensor.matmul(out=psa[:, :w], lhsT=n1b2r[:1, :],
                                     rhs=mrow[:1, :w],
                                     start=False, stop=True,
                                     skip_group_check=True)
                    aggs = sb.tile([P, 4 * P], f32, tag="aggs")
                    nc.vector.tensor_copy(out=aggs[:, :w], in_=psa[:, :w])

                    # node MLP for this chunk
                    psh = ps2.tile([P, 4 * P], f32, tag="h1")
                    nc.tensor.matmul(out=psh[:, :w], lhsT=w1x[:],
                                     rhs=xT[:, d0 * P:d0 * P + w],
                                     start=True, stop=False)
                    nc.tensor.matmul(out=psh[:, :w], lhsT=w1a[:],
                                     rhs=aggs[:, :w], start=False, stop=False,
                                     skip_group_check=True)
                    nc.tensor.matmul(out=psh[:, :w], lhsT=q[:],
                                     rhs=gbt[:, d0 * P:d0 * P + w],
                                     start=False, stop=True,
                                     skip_group_check=True)
                    h1r = sb.tile([P, 4 * P], f32, tag="h1r")
                    nc.scalar.activation(h1r[:, :w], psh[:, :w], AF.Relu,
                                         bias=n2b1c[:])
                    psx = ps1.tile([P, 4 * P], f32, tag="xnT")
                    nc.tensor.matmul(out=psx[:, :w], lhsT=n2w2[:],
                                     rhs=h1r[:, :w], start=True, stop=True)
                    nc.scalar.activation(xT[:, d0 * P:d0 * P + w], psx[:, :w],
                                         AF.Identity, bias=n2b2c[:])
                    for sub in range(ndt):
                        t = d0 + sub
                        nn = P if t < NDT - 1 else LAST_W
                        psn = ps1.tile([P, F], f32, tag="xnpz")
                        nc.tensor.matmul(out=psn[:nn, :],
                                         lhsT=h1r[:, sub * P:sub * P + nn],
                                         rhs=n2w2[:], start=True, stop=False)
                        nc.tensor.matmul(out=psn[:nn, :], lhsT=ones[:1, :nn],
                                         rhs=n2b2r[:], start=False, stop=True,
                                         skip_group_check=True)
                        xn = sb.tile([P, F], f32, tag="xn")
                        nc.vector.tensor_copy(out=xn[:nn, :], in_=psn[:nn, :])
                        nc.tensor.matmul(out=psu[:],
                                         lhsT=gb[:nn, t * B:(t + 1) * B],
                                         rhs=xn[:nn, :], start=(t == 0),
                                         stop=(t == NDT - 1),
                                         skip_group_check=True)
                        if l == L - 1:
                            nc.sync.dma_start(x_out[t * P:t * P + nn, :],
                                              xn[:nn, :])

                # ---- phase U: global MLP
                uss = sb.tile([B, F], f32, tag="uss")
                nc.vector.tensor_copy(out=uss[:], in_=psu[:])
                ar_in = dram.tile([B, F], f32, tag="ar_in")
                ar_out = dram.tile([B, F], f32, tag="ar_out")
                nc.sync.dma_start(ar_in[:], uss[:])
                nc.gpsimd.collective_compute(
                    "AllReduce", mybir.AluOpType.add,
                    replica_groups=[list(range(NCORE))],
                    ins=[ar_in.opt()], outs=[ar_out.opt()],
                )
                usr = sb.tile([B, F], f32, tag="usr")
                nc.sync.dma_start(usr[:], ar_out[:])
                pst = ps1.tile([P, 4 * P], f32, tag="xnT")
                nc.tensor.transpose(out=pst[:, :B], in_=usr[:],
                                    identity=ident[:B, :B])
                umT = sb.tile([P, B], f32, tag="umT")
                nc.vector.tensor_tensor(out=umT[:], in0=pst[:, :B],
                                        in1=invb[:], op=mybir.AluOpType.mult)
                psg = ps2.tile([P, 4 * P], f32, tag="h1")
                nc.tensor.matmul(out=psg[:, :B], lhsT=gw1u[:], rhs=uT[:],
                                 start=True, stop=False)
                nc.tensor.matmul(out=psg[:, :B], lhsT=gw1m[:], rhs=umT[:],
                                 start=False, stop=True, skip_group_check=True)
                ghr = sb.tile([P, B], f32, tag="ghr")
                nc.scalar.activation(ghr[:, :], psg[:, :B], AF.Relu,
                                     bias=gb1c[:])
                psn2 = ps1.tile([P, F], f32, tag="xnpz")
                nc.tensor.matmul(out=psn2[:B, :], lhsT=ghr[:], rhs=gw2[:],
                                 start=True, stop=False)
                nc.tensor.matmul(out=psn2[:B, :], lhsT=ones[:1, :B],
                                 rhs=gb2r[:], start=False, stop=True,
                                 skip_group_check=True)
                if l == L - 1:
                    un = sb.tile([B, U], f32, tag="un")
                    nc.vector.tensor_copy(out=un[:], in_=psn2[:B, :])
                    nc.sync.dma_start(u_out[:], un[:])
                psut = ps1.tile([P, 4 * P], f32, tag="agg")
                nc.tensor.matmul(out=psut[:, :B], lhsT=gw2[:], rhs=ghr[:],
                                 start=True, stop=True)
                nc.scalar.activation(uT[:, :], psut[:, :B], AF.Identity,
                                     bias=gb2c[:])
                if l < L - 1:
                    w1un = load(wts, n2w1_d[l + 1, 2 * P:3 * P, :], [P, H],
                                tag="w1u")
                    psq = ps1.tile([B, H], f32, tag="us")
                    nc.tensor.matmul(out=psq[:], lhsT=uT[:], rhs=w1un[:],
                                     start=True, stop=True)
                    nc.vector.tensor_copy(out=q[:], in_=psq[:])
    nc.compile()
    return nc


def _prep_core(row, col, k, T):
    m = (col >= k * NS) & (col < (k + 1) * NS)
    er = row[m].astype(np.int64)
    ec = (col[m] - k * NS).astype(np.int64)
    idx_parts, rc_parts = [], []
    for d in range(NDT):
        dm = ec >> 7 == d
        erd, ecd = er[dm], ec[dm]
        for h in range(0):
            pass
    # build per (super, half, dtile) in stream order
    idx_stream = np.zeros(NDT * 2 * T * P, np.int16)
    rc_stream = np.full(NDT * 2 * T * P, -1.0, np.float32)
    pos = 0
    for d0, ndt in SUPERS:
        for h in range(2):
            for d in range(d0, d0 + ndt):
                dm = (ec >> 7) == d
                erd, ecd = er[dm], ec[dm]
                hm = erd < HALF if h == 0 else erd >= HALF
                ei = erd[hm] - (0 if h == 0 else HALF)
                ri = (ecd[hm] & 127).astype(np.float32)
                n = len(ei)
                assert n <= T * P, (k, d, h, n)
                idx_stream[pos:pos + n] = ei.astype(np.int16)
                rc_stream[pos:pos + n] = ri
                pos += T * P
    NT = NDT * 2 * T
    wrap = idx_stream.reshape(NT * 8, 16).T  # [16, NT*8]
    idx_wrap = np.tile(wrap, (8, 1)).astype(np.int16)  # [128, NT*8]
    rc_T = np.ascontiguousarray(rc_stream.reshape(NT, P).T)  # [128, NT]
    cnt = np.bincount(ec, minlength=NS).astype(np.float32)
    invc = np.tile((1.0 / np.maximum(cnt, 1.0))[None, :], (P, 1))
    maskr = (cnt > 0).astype(np.float32)[None, :]
    return idx_wrap, rc_T, invc, maskr


def kernel(x, edge_index, u, batch,
           n1w1, n1b1, n1w2, n1b2, n2w1, n2b1, n2w2, n2b2,
           gw1, gb1, gw2, gb2):
    from concourse.bass_utils import run_bass_kernel_spmd

    x = np.asarray(x, np.float32)
    u = np.asarray(u, np.float32)
    row = np.asarray(edge_index[0], np.int64)
    col = np.asarray(edge_index[1], np.int64)
    batch_np = np.asarray(batch, np.int64)
    prm = {k: np.asarray(v, np.float32) for k, v in dict(
        n1w1=n1w1, n1b1=n1b1, n1w2=n1w2, n1b2=n1b2, n2w1=n2w1, n2b1=n2b1,
        n2w2=n2w2, n2b2=n2b2, gw1=gw1, gb1=gb1, gw2=gw2, gb2=gb2).items()}

    # uniform K-tiles per (core, dtile, half)
    T = 1
    for k in range(NCORE):
        m = (col >= k * NS) & (col < (k + 1) * NS)
        ec = col[m] - k * NS
        er = row[m]
        for h in range(2):
            hm = er < HALF if h == 0 else er >= HALF
            c = np.bincount(ec[hm] >> 7, minlength=NDT)
            T = max(T, int(-(-c.max() // P)))

    if T not in _BUILD_CACHE:
        _BUILD_CACHE[T] = _build(T)
    nc = _BUILD_CACHE[T]

    bcnt = np.bincount(batch_np, minlength=B).astype(np.float32)
    invb = np.tile((1.0 / np.maximum(bcnt, 1.0))[None, :], (P, 1))
    iota = np.tile(np.arange(P, dtype=np.float32), (P, 8, 1))
    common = dict(
        uT_in=np.ascontiguousarray(u.T), invb_in=invb, iota_in=iota,
        ones_in=np.ones((1, P), np.float32),
        ident_in=np.eye(P, dtype=np.float32),
        n1w1_d=prm["n1w1"], n1b1_d=prm["n1b1"][:, None, :],
        n1w2_d=prm["n1w2"], n1b2_d=prm["n1b2"][:, None, :],
        n2w1_d=prm["n2w1"], n2b1_d=prm["n2b1"][:, :, None],
        n2w2_d=prm["n2w2"], n2b2c_d=prm["n2b2"][:, :, None],
        n2b2r_d=prm["n2b2"][:, None, :],
        gw1_d=prm["gw1"], gb1_d=prm["gb1"][:, :, None],
        gw2_d=prm["gw2"], gb2c_d=prm["gb2"][:, :, None],
        gb2r_d=prm["gb2"][:, None, :],
    )
    in_maps = []
    for k in range(NCORE):
        idx_wrap, rc_T, invc, maskr = _prep_core(row, col, k, T)
        nsl = slice(k * NS, (k + 1) * NS)
        bloc = batch_np[nsl]
        gb_arr = np.zeros((P, NDT * B), np.float32)
        gbt_arr = np.zeros((B, NS), np.float32)
        for t in range(NDT):
            nn = P if t < NDT - 1 else LAST_W
            bb = bloc[t * P:t * P + nn]
            gb_arr[np.arange(nn), t * B + bb] = 1.0
        gbt_arr[bloc, np.arange(NS)] = 1.0
        in_maps.append(dict(
            xT_in=np.ascontiguousarray(x[nsl].T), idx_in=idx_wrap,
            rc_in=rc_T, invc_in=invc, mask_in=maskr, gb_in=gb_arr,
            gbt_in=gbt_arr, **common))

    global LAST_RESULT, LAST_EXEC_NS
    import time as _t
    _t0 = _t.perf_counter()
    res = run_bass_kernel_spmd(nc, in_maps, core_ids=list(range(NCORE)))
    LAST_EXEC_NS = int((_t.perf_counter() - _t0) * 1e9)
    LAST_RESULT = res
    x_new = np.concatenate([res.results[k]["x_out"] for k in range(NCORE)], 0)
    u_new = res.results[0]["u_out"]
    return (x_new, u_new)
